# revision 13
# baseline (speedup 1.0000x reference)
"""Trainium2 Bass kernel for a small MLP: [N,2] -> 32 -> (8x 32) -> 1.

Strategy (data-parallel over 8 cores, batch-sharded):
  - Per core R=262144 rows, processed in 32 supertiles of 8192 rows.
  - A supertile lives in SBUF as [128 partitions, 2048 free]: 4 partition
    blocks (32 hidden channels each) x 4 free blocks (512 rows each) = 16
    groups of 512 batch rows. Group (i,f) = rows s*8192+(4i+f)*512+[0,512).
  - Each layer = 4 matmuls of [K,128]x[K,512] with BLOCK-DIAGONAL weights:
    one instruction advances 4 groups (2048 batch rows) in 512 moving rows.
    float32r end-to-end (DRAM/SBUF tiles typed f32r; drains round to f32r)
    for the 1 cycle/row PE fast path.
  - bias+ReLU drain PSUM->SBUF split across ACT / DVE / Pool by rate.
  - Output layer: Wout replicated x32 -> every PSUM partition holds y;
    copy-drain, DMA partitions {0,32,64,96} to DRAM; bout added on host.
"""

import numpy as np

N = 2097152
H = 32
L = 8
N_CORES = 8
R = N // N_CORES          # 262144 rows per core
FB = 512                  # rows per group
ST_ROWS = 16 * FB         # 8192 rows per supertile
N_ST = R // ST_ROWS       # 32 supertiles per core

# Drain split (free columns of the 2048-wide supertile per engine),
# proportional to engine rates ACT 1.2 / DVE 0.96 cols/ns. GPSIMD (Pool)
# cannot access PSUM on TRN2, so drains are ACT+DVE only.
ACT_COLS = 1136
DVE_COLS = 2048 - ACT_COLS

_CACHE = {}


def _build_nc(n_st=N_ST):
    import concourse.tile as tile
    from concourse import bacc, mybir

    f32 = mybir.dt.float32
    f32r = mybir.dt.float32r

    nc = bacc.Bacc(None, target_bir_lowering=False)
    xt_d = nc.dram_tensor("xt", [8, n_st, 2048], f32r, kind="ExternalInput")
    wm_d = nc.dram_tensor("wmat", [128, 1280], f32r, kind="ExternalInput")
    wb_d = nc.dram_tensor("wbias", [128, 9], f32, kind="ExternalInput")
    out_d = nc.dram_tensor("out", [n_st, 4, 2048], f32, kind="ExternalOutput")

    relu = mybir.ActivationFunctionType.Relu
    alu_add = mybir.AluOpType.add
    alu_max = mybir.AluOpType.max

    with tile.TileContext(nc) as tc:
        with tc.tile_pool(name="wpool", bufs=1) as wpool, \
             tc.tile_pool(name="xpool", bufs=3) as xpool, \
             tc.tile_pool(name="hpool", bufs=4) as hpool, \
             tc.tile_pool(name="pspool", bufs=2, space="PSUM") as pspool:
            w = wpool.tile([128, 1280], f32r)
            nc.sync.dma_start(out=w[:], in_=wm_d[:, :])
            wb = wpool.tile([128, 9], f32)
            nc.sync.dma_start(out=wb[:], in_=wb_d[:, :])

            for s in range(n_st):
                x_t = xpool.tile([8, 2048], f32r)
                nc.sync.dma_start(out=x_t[:], in_=xt_d[:, s, :])
                h_prev, kdim = x_t, 8
                for l in range(10):
                    ps = pspool.tile([128, 2048], f32)
                    for f in range(4):
                        nc.tensor.matmul(
                            ps[:, 512 * f:512 * (f + 1)],
                            w[0:kdim, 128 * l:128 * (l + 1)],
                            h_prev[0:kdim, 512 * f:512 * (f + 1)])
                    if l == 9:
                        ho = hpool.tile([128, 2048], f32)
                        c0, c1 = 0, ACT_COLS
                        nc.scalar.copy(ho[:, c0:c1], ps[:, c0:c1])
                        c0, c1 = c1, 2048
                        nc.vector.tensor_scalar_add(ho[:, c0:c1],
                                                    ps[:, c0:c1], 0.0)
                        src = ho[:].rearrange("(i r) f -> i r f",
                                              r=32)[:, 0, :]
                        nc.sync.dma_start(out=out_d[s, :, :], in_=src)
                    else:
                        hn = hpool.tile([128, 2048], f32r)
                        bias = wb[:, l:l + 1]
                        c0, c1 = 0, ACT_COLS
                        nc.scalar.activation(hn[:, c0:c1], ps[:, c0:c1],
                                             relu, bias=bias)
                        c0, c1 = c1, 2048
                        nc.vector.tensor_scalar(hn[:, c0:c1], ps[:, c0:c1],
                                                bias, 0.0, alu_add, alu_max)
                        h_prev, kdim = hn, 128
    nc.finalize()
    return nc


def _prep_core_inputs(x_shard, wmat, wbias):
    # xt[2i+c, s, 512f+r] = x_shard[s*8192 + (4i+f)*512 + r, c]
    xs = np.ascontiguousarray(x_shard, dtype=np.float32).reshape(
        N_ST, 4, 4, FB, 2)
    xt = np.ascontiguousarray(xs.transpose(1, 4, 0, 2, 3)).reshape(
        8, N_ST, 2048)
    return {"xt": xt, "wmat": wmat, "wbias": wbias}


def _pack_weights(W0, b0, Wh, bh, Wout):
    # Block-diagonal lhsT per layer, 128 cols each:
    #   l=0:    wmat[2i+c, 32i+m]       = W0[m, c]        (K=8 rows used)
    #   l=1..8: wmat[32i+k, 128l+32i+m] = Wh[l-1][m, k]
    #   l=9:    wmat[32i+k, 1152+32i+m] = Wout[0, k]      (replicated x32)
    wmat = np.zeros((128, 1280), dtype=np.float32)
    wbias = np.zeros((128, 9), dtype=np.float32)
    for i in range(4):
        wmat[2 * i:2 * i + 2, 32 * i:32 * i + 32] = W0.T
        for hl in range(L):
            wmat[32 * i:32 * i + 32,
                 128 * (hl + 1) + 32 * i:128 * (hl + 1) + 32 * i + 32] = \
                Wh[hl].T
        wmat[32 * i:32 * i + 32, 1152 + 32 * i:1152 + 32 * i + 32] = \
            Wout[0, :, None]
        wbias[32 * i:32 * i + 32, 0] = b0
        for hl in range(L):
            wbias[32 * i:32 * i + 32, 1 + hl] = bh[hl]
    return wmat, wbias


def kernel(x, W0, b0, Wh, bh, Wout, bout):
    from concourse import bass_utils

    if "nc" not in _CACHE:
        _CACHE["nc"] = _build_nc()
    nc = _CACHE["nc"]

    wmat, wbias = _pack_weights(np.asarray(W0, np.float32),
                                np.asarray(b0, np.float32),
                                np.asarray(Wh, np.float32),
                                np.asarray(bh, np.float32),
                                np.asarray(Wout, np.float32))
    x = np.asarray(x, np.float32)
    in_maps = [_prep_core_inputs(x[c * R:(c + 1) * R], wmat, wbias)
               for c in range(N_CORES)]

    res = bass_utils.run_bass_kernel_spmd(nc, in_maps, list(range(N_CORES)))
    out = np.concatenate([r["out"].reshape(R) for r in res.results])
    return (out.reshape(N, 1) + np.float32(bout[0])).astype(np.float32)


# revision 14
# speedup vs baseline: 1.7720x; 1.7720x over previous
"""Trainium2 Bass kernel for a small MLP: [N,2] -> 32 -> (8x 32) -> 1.

Strategy (data-parallel over 8 cores, batch-sharded):
  - Per core R=262144 rows, processed in 32 supertiles of 8192 rows.
  - A supertile lives in SBUF as [128 partitions, 2048 free]: 4 partition
    blocks (32 hidden channels each) x 4 free blocks (512 rows each) = 16
    groups of 512 batch rows. Group (i,f) = rows s*8192+(4i+f)*512+[0,512).
  - Each layer = 4 matmuls of [K,128]x[K,512] with BLOCK-DIAGONAL weights:
    one instruction advances 4 groups (2048 batch rows) in 512 moving rows.
    float32r end-to-end (DRAM/SBUF tiles typed f32r; drains round to f32r)
    for the 1 cycle/row PE fast path.
  - bias+ReLU drain PSUM->SBUF split across ACT / DVE / Pool by rate.
  - Output layer: Wout replicated x32 -> every PSUM partition holds y;
    copy-drain, DMA partitions {0,32,64,96} to DRAM; bout added on host.
"""

import numpy as np

N = 2097152
H = 32
L = 8
N_CORES = 8
R = N // N_CORES          # 262144 rows per core
FB = 512                  # rows per group
ST_ROWS = 16 * FB         # 8192 rows per supertile
N_ST = R // ST_ROWS       # 32 supertiles per core

# Drain split (free columns of the 2048-wide supertile per engine),
# proportional to engine rates ACT 1.2 / DVE 0.96 cols/ns. GPSIMD (Pool)
# cannot access PSUM on TRN2, so drains are ACT+DVE only.
ACT_COLS = 1136
DVE_COLS = 2048 - ACT_COLS

_CACHE = {}


def _build_nc(n_st=N_ST):
    import concourse.tile as tile
    from concourse import bacc, mybir

    f32 = mybir.dt.float32
    f32r = mybir.dt.float32r

    nc = bacc.Bacc(None, target_bir_lowering=False)
    xt_d = nc.dram_tensor("xt", [8, n_st, 2048], f32r, kind="ExternalInput")
    wm_d = nc.dram_tensor("wmat", [128, 1280], f32r, kind="ExternalInput")
    wb_d = nc.dram_tensor("wbias", [128, 9], f32, kind="ExternalInput")
    out_d = nc.dram_tensor("out", [n_st, 4, 2048], f32, kind="ExternalOutput")

    relu = mybir.ActivationFunctionType.Relu
    alu_add = mybir.AluOpType.add
    alu_max = mybir.AluOpType.max

    with tile.TileContext(nc) as tc:
        with tc.tile_pool(name="wpool", bufs=1) as wpool, \
             tc.tile_pool(name="xpool", bufs=4) as xpool, \
             tc.tile_pool(name="hpool", bufs=4) as hpool, \
             tc.tile_pool(name="pspool", bufs=2, space="PSUM") as pspool:
            w = wpool.tile([128, 1280], f32r)
            nc.sync.dma_start(out=w[:], in_=wm_d[:, :])
            wb = wpool.tile([128, 9], f32)
            nc.sync.dma_start(out=wb[:], in_=wb_d[:, :])

            def layer(s, l, h_prev, kdim):
                ps = pspool.tile([128, 2048], f32)
                for f in range(4):
                    nc.tensor.matmul(
                        ps[:, 512 * f:512 * (f + 1)],
                        w[0:kdim, 128 * l:128 * (l + 1)],
                        h_prev[0:kdim, 512 * f:512 * (f + 1)])
                if l == 9:
                    ho = hpool.tile([128, 2048], f32)
                    c0, c1 = 0, ACT_COLS
                    nc.scalar.copy(ho[:, c0:c1], ps[:, c0:c1])
                    c0, c1 = c1, 2048
                    nc.vector.tensor_scalar_add(ho[:, c0:c1],
                                                ps[:, c0:c1], 0.0)
                    src = ho[:].rearrange("(i r) f -> i r f", r=32)[:, 0, :]
                    nc.sync.dma_start(out=out_d[s, :, :], in_=src)
                    return None
                hn = hpool.tile([128, 2048], f32r)
                bias = wb[:, l:l + 1]
                c0, c1 = 0, ACT_COLS
                nc.scalar.activation(hn[:, c0:c1], ps[:, c0:c1],
                                     relu, bias=bias)
                c0, c1 = c1, 2048
                nc.vector.tensor_scalar(hn[:, c0:c1], ps[:, c0:c1],
                                        bias, 0.0, alu_add, alu_max)
                return hn

            # Two supertiles in flight: while supertile A's layer-l drain
            # runs on ACT/DVE, the PE does supertile B's layer-l matmuls.
            # Removes the per-layer PE stall and keeps the PE continuously
            # busy (p-state ramp needs 3us of uninterrupted execution).
            for t in range(n_st // 2):
                sa, sb = 2 * t, 2 * t + 1
                xa = xpool.tile([8, 2048], f32r)
                nc.sync.dma_start(out=xa[:], in_=xt_d[:, sa, :])
                xb = xpool.tile([8, 2048], f32r)
                nc.sync.dma_start(out=xb[:], in_=xt_d[:, sb, :])
                ha, hb, kdim = xa, xb, 8
                for l in range(10):
                    ha2 = layer(sa, l, ha, kdim)
                    hb2 = layer(sb, l, hb, kdim)
                    ha, hb, kdim = ha2, hb2, 128
    nc.finalize()
    return nc


def _prep_core_inputs(x_shard, wmat, wbias):
    # xt[2i+c, s, 512f+r] = x_shard[s*8192 + (4i+f)*512 + r, c]
    xs = np.ascontiguousarray(x_shard, dtype=np.float32).reshape(
        N_ST, 4, 4, FB, 2)
    xt = np.ascontiguousarray(xs.transpose(1, 4, 0, 2, 3)).reshape(
        8, N_ST, 2048)
    return {"xt": xt, "wmat": wmat, "wbias": wbias}


def _pack_weights(W0, b0, Wh, bh, Wout):
    # Block-diagonal lhsT per layer, 128 cols each:
    #   l=0:    wmat[2i+c, 32i+m]       = W0[m, c]        (K=8 rows used)
    #   l=1..8: wmat[32i+k, 128l+32i+m] = Wh[l-1][m, k]
    #   l=9:    wmat[32i+k, 1152+32i+m] = Wout[0, k]      (replicated x32)
    wmat = np.zeros((128, 1280), dtype=np.float32)
    wbias = np.zeros((128, 9), dtype=np.float32)
    for i in range(4):
        wmat[2 * i:2 * i + 2, 32 * i:32 * i + 32] = W0.T
        for hl in range(L):
            wmat[32 * i:32 * i + 32,
                 128 * (hl + 1) + 32 * i:128 * (hl + 1) + 32 * i + 32] = \
                Wh[hl].T
        wmat[32 * i:32 * i + 32, 1152 + 32 * i:1152 + 32 * i + 32] = \
            Wout[0, :, None]
        wbias[32 * i:32 * i + 32, 0] = b0
        for hl in range(L):
            wbias[32 * i:32 * i + 32, 1 + hl] = bh[hl]
    return wmat, wbias


def kernel(x, W0, b0, Wh, bh, Wout, bout):
    from concourse import bass_utils

    if "nc" not in _CACHE:
        _CACHE["nc"] = _build_nc()
    nc = _CACHE["nc"]

    wmat, wbias = _pack_weights(np.asarray(W0, np.float32),
                                np.asarray(b0, np.float32),
                                np.asarray(Wh, np.float32),
                                np.asarray(bh, np.float32),
                                np.asarray(Wout, np.float32))
    x = np.asarray(x, np.float32)
    in_maps = [_prep_core_inputs(x[c * R:(c + 1) * R], wmat, wbias)
               for c in range(N_CORES)]

    res = bass_utils.run_bass_kernel_spmd(nc, in_maps, list(range(N_CORES)))
    out = np.concatenate([r["out"].reshape(R) for r in res.results])
    return (out.reshape(N, 1) + np.float32(bout[0])).astype(np.float32)


# revision 18
# speedup vs baseline: 1.7965x; 1.0138x over previous
"""Trainium2 Bass kernel for a small MLP: [N,2] -> 32 -> (8x 32) -> 1.

Strategy (data-parallel over 8 cores, batch-sharded):
  - Per core R=262144 rows, processed in 32 supertiles of 8192 rows.
  - A supertile lives in SBUF as [128 partitions, 2048 free]: 4 partition
    blocks (32 hidden channels each) x 4 free blocks (512 rows each) = 16
    groups of 512 batch rows. Group (i,f) = rows s*8192+(4i+f)*512+[0,512).
  - Each layer = 4 matmuls of [K,128]x[K,512] with BLOCK-DIAGONAL weights:
    one instruction advances 4 groups (2048 batch rows) in 512 moving rows.
    float32r end-to-end (DRAM/SBUF tiles typed f32r; drains round to f32r)
    for the 1 cycle/row PE fast path.
  - bias+ReLU drain PSUM->SBUF split across ACT / DVE / Pool by rate.
  - Output layer: Wout replicated x32 -> every PSUM partition holds y;
    copy-drain, DMA partitions {0,32,64,96} to DRAM; bout added on host.
"""

import numpy as np

N = 2097152
H = 32
L = 8
N_CORES = 8
R = N // N_CORES          # 262144 rows per core
FB = 512                  # rows per group
ST_ROWS = 16 * FB         # 8192 rows per supertile
N_ST = R // ST_ROWS       # 32 supertiles per core

# Drain split (free columns of the 2048-wide supertile per engine),
# proportional to engine rates ACT 1.2 / DVE 0.96 cols/ns. GPSIMD (Pool)
# cannot access PSUM on TRN2, so drains are ACT+DVE only.
ACT_COLS = 1136
DVE_COLS = 2048 - ACT_COLS

_CACHE = {}


def _build_nc(n_st=N_ST):
    import concourse.tile as tile
    from concourse import bacc, mybir

    f32 = mybir.dt.float32
    f32r = mybir.dt.float32r

    nc = bacc.Bacc(None, target_bir_lowering=False)
    xt_d = nc.dram_tensor("xt", [8, n_st, 2048], f32r, kind="ExternalInput")
    wm_d = nc.dram_tensor("wmat", [128, 1280], f32r, kind="ExternalInput")
    wb_d = nc.dram_tensor("wbias", [128, 9], f32, kind="ExternalInput")
    out_d = nc.dram_tensor("out", [n_st, 4, 2048], f32, kind="ExternalOutput")

    relu = mybir.ActivationFunctionType.Relu
    alu_add = mybir.AluOpType.add
    alu_max = mybir.AluOpType.max

    with tile.TileContext(nc) as tc:
        with tc.tile_pool(name="wpool", bufs=1) as wpool, \
             tc.tile_pool(name="xpool", bufs=4) as xpool, \
             tc.tile_pool(name="hpool", bufs=4) as hpool, \
             tc.tile_pool(name="pspool", bufs=2, space="PSUM") as pspool:
            w = wpool.tile([128, 1280], f32r)
            nc.sync.dma_start(out=w[:], in_=wm_d[:, :])
            wb = wpool.tile([128, 9], f32)
            nc.sync.dma_start(out=wb[:], in_=wb_d[:, :])

            def layer(s, l, h_prev, kdim):
                ps = pspool.tile([128, 2048], f32)
                for f in range(4):
                    nc.tensor.matmul(
                        ps[:, 512 * f:512 * (f + 1)],
                        w[0:kdim, 128 * l:128 * (l + 1)],
                        h_prev[0:kdim, 512 * f:512 * (f + 1)])
                if l == 9:
                    ho = hpool.tile([128, 2048], f32)
                    c0, c1 = 0, ACT_COLS
                    nc.scalar.copy(ho[:, c0:c1], ps[:, c0:c1])
                    c0, c1 = c1, 2048
                    nc.vector.tensor_scalar_add(ho[:, c0:c1],
                                                ps[:, c0:c1], 0.0)
                    src = ho[:].rearrange("(i r) f -> i r f", r=32)[:, 0, :]
                    nc.sync.dma_start(out=out_d[s, :, :], in_=src)
                    return None
                hn = hpool.tile([128, 2048], f32r)
                bias = wb[:, l:l + 1]
                c0, c1 = 0, ACT_COLS
                nc.scalar.activation(hn[:, c0:c1], ps[:, c0:c1],
                                     relu, bias=bias)
                c0, c1 = c1, 2048
                nc.vector.tensor_scalar(hn[:, c0:c1], ps[:, c0:c1],
                                        bias, 0.0, alu_add, alu_max)
                return hn

            # Four supertiles in flight: while supertile A's layer-l drain
            # runs on ACT/DVE, the PE does B/C/D's layer-l matmuls. Removes
            # per-layer PE stalls and keeps the PE continuously busy
            # (p-state ramp needs 3us of uninterrupted execution).
            for t in range(n_st // 4):
                ss = [4 * t + j for j in range(4)]
                hs = []
                for s in ss:
                    xv = xpool.tile([8, 2048], f32r)
                    nc.sync.dma_start(out=xv[:], in_=xt_d[:, s, :])
                    hs.append(xv)
                kdim = 8
                for l in range(10):
                    hs = [layer(ss[j], l, hs[j], kdim) for j in range(4)]
                    kdim = 128
    nc.finalize()
    return nc


def _prep_core_inputs(x_shard, wmat, wbias):
    # xt[2i+c, s, 512f+r] = x_shard[s*8192 + (4i+f)*512 + r, c]
    xs = np.ascontiguousarray(x_shard, dtype=np.float32).reshape(
        N_ST, 4, 4, FB, 2)
    xt = np.ascontiguousarray(xs.transpose(1, 4, 0, 2, 3)).reshape(
        8, N_ST, 2048)
    return {"xt": xt, "wmat": wmat, "wbias": wbias}


def _pack_weights(W0, b0, Wh, bh, Wout):
    # Block-diagonal lhsT per layer, 128 cols each:
    #   l=0:    wmat[2i+c, 32i+m]       = W0[m, c]        (K=8 rows used)
    #   l=1..8: wmat[32i+k, 128l+32i+m] = Wh[l-1][m, k]
    #   l=9:    wmat[32i+k, 1152+32i+m] = Wout[0, k]      (replicated x32)
    wmat = np.zeros((128, 1280), dtype=np.float32)
    wbias = np.zeros((128, 9), dtype=np.float32)
    for i in range(4):
        wmat[2 * i:2 * i + 2, 32 * i:32 * i + 32] = W0.T
        for hl in range(L):
            wmat[32 * i:32 * i + 32,
                 128 * (hl + 1) + 32 * i:128 * (hl + 1) + 32 * i + 32] = \
                Wh[hl].T
        wmat[32 * i:32 * i + 32, 1152 + 32 * i:1152 + 32 * i + 32] = \
            Wout[0, :, None]
        wbias[32 * i:32 * i + 32, 0] = b0
        for hl in range(L):
            wbias[32 * i:32 * i + 32, 1 + hl] = bh[hl]
    return wmat, wbias


def kernel(x, W0, b0, Wh, bh, Wout, bout):
    from concourse import bass_utils

    if "nc" not in _CACHE:
        _CACHE["nc"] = _build_nc()
    nc = _CACHE["nc"]

    wmat, wbias = _pack_weights(np.asarray(W0, np.float32),
                                np.asarray(b0, np.float32),
                                np.asarray(Wh, np.float32),
                                np.asarray(bh, np.float32),
                                np.asarray(Wout, np.float32))
    x = np.asarray(x, np.float32)
    in_maps = [_prep_core_inputs(x[c * R:(c + 1) * R], wmat, wbias)
               for c in range(N_CORES)]

    res = bass_utils.run_bass_kernel_spmd(nc, in_maps, list(range(N_CORES)))
    out = np.concatenate([r["out"].reshape(R) for r in res.results])
    return (out.reshape(N, 1) + np.float32(bout[0])).astype(np.float32)
